# revision 3
# baseline (speedup 1.0000x reference)
"""Trainium2 Bass kernel for causal GQA attention (B=2, S=2048, D=2048,
H=32, KVH=8, hd=64) with RoPE and output projection, running SPMD on 8
NeuronCores.

Sharding: tensor-parallel over heads (4-way) x data-parallel over batch
(2-way).  Core c (b = c//4, k = c%4) handles batch b and heads
8k..8k+8 (kv heads 2k, 2k+1).  Attention outputs are AllGathered within
each batch group of 4 cores, after which each core computes a 512-wide
output-dim slice of the wo projection.  The host assembles the full
output, so no AllReduce is needed.

Layouts: everything lives in transposed [feature, seq] form so that the
head dim (the contraction dim of QK^T) sits on SBUF partitions and no
on-device transposes are required (except a cheap PE transpose for V).
All matmuls run in float32r (fp32 storage, ~tf32 rounding, 1 cyc/row).
"""

import numpy as np

DIM = 2048
S = 2048
B = 2
H = 32
KVH = 8
HD = 64
P = 128
HL = 8          # heads per core
QT = 512        # q tile (free dim of score matmuls)
NQT = S // QT   # 4
NKV = S // P    # 16 kv tiles of 128
DK = DIM // P   # 16 contraction tiles
ROPE_BASE = 10000.0
N_CORES = 8

_CACHE = {}


def _build():
    import concourse.bacc as bacc
    import concourse.tile as tile
    import concourse.mybir as mybir
    from concourse.masks import make_identity

    F32 = mybir.dt.float32
    F32R = mybir.dt.float32r
    Exp = mybir.ActivationFunctionType.Exp

    nc = bacc.Bacc("TRN2", target_bir_lowering=False, debug=False,
                   num_devices=N_CORES)

    xT = nc.dram_tensor("xT", [DIM, S], F32R, kind="ExternalInput").ap()
    wqT = nc.dram_tensor("wqT", [DIM, 512], F32R, kind="ExternalInput").ap()
    wkT = nc.dram_tensor("wkT", [DIM, 256], F32R, kind="ExternalInput").ap()
    wvT = nc.dram_tensor("wvT", [DIM, 128], F32R, kind="ExternalInput").ap()
    woT = nc.dram_tensor("woT", [DIM, 512], F32R, kind="ExternalInput").ap()
    cosT = nc.dram_tensor("cosT", [P, S], F32, kind="ExternalInput").ap()
    sinT = nc.dram_tensor("sinT", [P, S], F32, kind="ExternalInput").ap()
    maskT = nc.dram_tensor("maskT", [P, 4, QT], F32, kind="ExternalInput").ap()
    out_t = nc.dram_tensor("out_t", [512, S], F32, kind="ExternalOutput").ap()

    xT3 = xT.rearrange("(o p) s -> p o s", p=P)
    wqT3 = wqT.rearrange("(o p) f -> p o f", p=P)
    wkT3 = wkT.rearrange("(o p) f -> p o f", p=P)
    wvT3 = wvT.rearrange("(o p) f -> p o f", p=P)
    woT3 = woT.rearrange("(o p) f -> p o f", p=P)

    with tile.TileContext(nc) as tc:
        with (
            tc.tile_pool(name="pers", bufs=1) as pers,
            tc.tile_pool(name="ps", bufs=1, space="PSUM") as ps,
            tc.tile_pool(name="dram", bufs=1, space="DRAM") as dram,
        ):
            # ---- persistent tiles ----
            q_fin = [pers.tile([P, S], F32R, name=f"q_fin{m}") for m in range(4)]
            k_fin = [pers.tile([P, S], F32R, name=f"k_fin{g}") for g in range(2)]
            v1 = [pers.tile([P, NKV, P], F32R, name=f"v1_{g}") for g in range(2)]
            msk = pers.tile([P, 4, QT], F32, name="msk")
            nc.sync.dma_start(msk[:], maskT[:])

            cc_in = dram.tile([512, S], F32R)
            cc_out = dram.tile([4 * 512, S], F32R)

            def bank(i):
                return ps.tile([P, QT], F32, tag=f"bank{i}", name=f"psum_b{i}")

            # ================= stage A/B: projections + RoPE =================
            with tc.tile_pool(name="pa", bufs=1) as pa:
                wq_sb = pa.tile([P, DK, 512], F32R)
                wk_sb = pa.tile([P, DK, 256], F32R)
                wv_sb = pa.tile([P, DK, 128], F32R)
                cos_sb = pa.tile([P, S], F32)
                sin_sb = pa.tile([P, S], F32)
                ident_f = pa.tile([P, P], F32)
                ident = pa.tile([P, P], F32R)
                nc.sync.dma_start(wq_sb[:], wqT3[:])
                nc.sync.dma_start(wk_sb[:], wkT3[:])
                nc.sync.dma_start(wv_sb[:], wvT3[:])
                nc.sync.dma_start(cos_sb[:], cosT[:])
                nc.sync.dma_start(sin_sb[:], sinT[:])
                make_identity(nc, ident_f[:])
                nc.vector.tensor_copy(ident[:], ident_f[:])

                vT_raw = pa.tile([P, S], F32R)

                # ones columns of the PV stationary operand (memset on f32r
                # is not a valid ISA encoding, so memset f32 then copy)
                ones3 = pa.tile([P, NKV, HD], F32)
                nc.vector.memset(ones3[:], 1.0)
                for g in range(2):
                    nc.vector.tensor_copy(v1[g][:, :, 0:HD], ones3[:])

                OCH = 4  # contraction chunk (in 128-tiles) per DMA
                for st in range(NQT):
                    ssl = slice(st * QT, (st + 1) * QT)
                    psq = [bank(m) for m in range(4)]
                    psk = [bank(4 + g) for g in range(2)]
                    psv = bank(6)
                    for oc in range(DK // OCH):
                        xsl = pa.tile([P, OCH, QT], F32R, tag="xsl", bufs=3,
                                      name="xsl")
                        nc.sync.dma_start(
                            xsl[:], xT3[:, oc * OCH:(oc + 1) * OCH, ssl])
                        for oo in range(OCH):
                            o = oc * OCH + oo
                            first = o == 0
                            last = o == DK - 1
                            for m in range(4):
                                nc.tensor.matmul(
                                    psq[m][:],
                                    wq_sb[:, o, m * P:(m + 1) * P],
                                    xsl[:, oo, :],
                                    start=first, stop=last)
                            for g in range(2):
                                nc.tensor.matmul(
                                    psk[g][:],
                                    wk_sb[:, o, g * P:(g + 1) * P],
                                    xsl[:, oo, :],
                                    start=first, stop=last)
                            nc.tensor.matmul(
                                psv[:], wv_sb[:, o, :], xsl[:, oo, :],
                                start=first, stop=last)

                    # RoPE on q/k slices; V^T raw copy
                    for i, (dst, src) in enumerate(
                            [(q_fin[m], psq[m]) for m in range(4)]
                            + [(k_fin[g], psk[g]) for g in range(2)]):
                        raw = pa.tile([P, QT], F32, tag="raw", bufs=3,
                                      name="raw")
                        nc.vector.tensor_copy(raw[:], src[:])
                        rot = pa.tile([P, QT], F32, tag="rot", bufs=3,
                                      name="rot")
                        for hh in range(2):
                            base = hh * HD
                            nc.sync.dma_start(rot[base:base + 32, :],
                                              raw[base + 32:base + 64, :])
                            nc.sync.dma_start(rot[base + 32:base + 64, :],
                                              raw[base:base + 32, :])
                        nc.vector.tensor_mul(rot[:], rot[:], sin_sb[:, ssl])
                        nc.vector.tensor_mul(raw[:], raw[:], cos_sb[:, ssl])
                        nc.vector.tensor_add(dst[:, ssl], raw[:], rot[:])
                    nc.vector.tensor_copy(vT_raw[:, ssl], psv[:])

                # V1 assembly: transpose vT_raw 128x128 blocks
                for j in range(NKV):
                    pst = ps.tile([P, P], F32R, tag="bank7", name="psum_tr")
                    nc.tensor.transpose(pst[:], vT_raw[:, j * P:(j + 1) * P],
                                        ident[:])
                    for g in range(2):
                        nc.vector.tensor_copy(
                            v1[g][:, j, HD:P], pst[:, g * HD:(g + 1) * HD])

            # ================= stage C: attention =================
            with tc.tile_pool(name="pc", bufs=1) as pc:
                for t in range(NQT):
                    nkv = 4 * (t + 1)
                    qsl = slice(t * QT, (t + 1) * QT)
                    for h in range(HL):
                        m, half, g = h // 2, h % 2, h // 4
                        prange = slice(half * HD, half * HD + HD)
                        pspv = bank(4 + (h % 2))
                        e_tiles = []
                        for j in range(nkv):
                            pss = bank(j % 3)
                            nc.tensor.matmul(
                                pss[:],
                                k_fin[g][prange, j * P:(j + 1) * P],
                                q_fin[m][prange, qsl],
                                start=True, stop=True)
                            e_j = pc.tile([P, QT], F32R, tag="exp", bufs=8,
                                          name="e_j")
                            nc.scalar.activation(e_j[:], pss[:], Exp,
                                                 scale=0.125)
                            c = j - 4 * t
                            if c >= 0:
                                nc.vector.tensor_mul(e_j[:], e_j[:],
                                                     msk[:, c, :])
                            e_tiles.append(e_j)
                        for j in range(nkv):
                            nc.tensor.matmul(
                                pspv[:], v1[g][:, j, :], e_tiles[j][:],
                                start=(j == 0), stop=(j == nkv - 1))
                        recip = pc.tile([1, QT], F32, tag="recip", bufs=2,
                                        name="recip")
                        nc.vector.reciprocal(recip[:], pspv[0:1, :])
                        bcast = pc.tile([P, QT], F32, tag="bcast", bufs=2,
                                        name="bcast")
                        nc.gpsimd.partition_broadcast(bcast[:], recip[:])
                        o_sb = pc.tile([P, QT], F32R, tag="osb", bufs=2,
                                       name="o_sb")
                        nc.vector.tensor_mul(o_sb[HD:P, :], pspv[HD:P, :],
                                             bcast[HD:P, :])
                        nc.sync.dma_start(cc_in[h * HD:(h + 1) * HD, qsl],
                                          o_sb[HD:P, :])

            # ================= stage D: AllGather =================
            nc.gpsimd.collective_compute(
                "AllGather",
                mybir.AluOpType.bypass,
                replica_groups=[[0, 1, 2, 3], [4, 5, 6, 7]],
                ins=[cc_in[:].opt()],
                outs=[cc_out[:].opt()],
            )
            cc3 = cc_out[:].rearrange("(o p) s -> p o s", p=P)

            # ================= stage E: wo projection =================
            with tc.tile_pool(name="pe", bufs=1) as pe:
                wo_sb = pe.tile([P, DK, 512], F32R)
                nc.sync.dma_start(wo_sb[:], woT3[:])
                OCH = 4
                for st in range(NQT):
                    ssl = slice(st * QT, (st + 1) * QT)
                    pso = [bank(d) for d in range(4)]
                    for oc in range(DK // OCH):
                        csl = pe.tile([P, OCH, QT], F32R, tag="csl", bufs=3,
                                      name="csl")
                        nc.sync.dma_start(
                            csl[:], cc3[:, oc * OCH:(oc + 1) * OCH, ssl])
                        for oo in range(OCH):
                            o = oc * OCH + oo
                            for d in range(4):
                                nc.tensor.matmul(
                                    pso[d][:],
                                    wo_sb[:, o, d * P:(d + 1) * P],
                                    csl[:, oo, :],
                                    start=(o == 0), stop=(o == DK - 1))
                    for d in range(4):
                        ot = pe.tile([P, QT], F32, tag="ot", bufs=3, name="ot")
                        nc.vector.tensor_copy(ot[:], pso[d][:])
                        nc.sync.dma_start(out_t[d * P:(d + 1) * P, ssl], ot[:])

    nc.compile()
    return nc


def _prep_inputs(x, position_ids, wq, wk, wv, wo):
    x = np.asarray(x, dtype=np.float32)
    pos = np.asarray(position_ids).reshape(-1).astype(np.int64)
    wqTf = np.asarray(wq, dtype=np.float32).T
    wkTf = np.asarray(wk, dtype=np.float32).T
    wvTf = np.asarray(wv, dtype=np.float32).T
    woTf = np.asarray(wo, dtype=np.float32).T

    inv = 1.0 / (ROPE_BASE ** (np.arange(0, HD, 2, dtype=np.float32) / HD))
    freqs = np.outer(pos.astype(np.float32), inv)  # [S, 32]
    pidx = np.arange(P) % 32
    sign = np.where((np.arange(P) % HD) < 32, -1.0, 1.0).astype(np.float32)
    cosT = np.ascontiguousarray(np.cos(freqs)[:, pidx].T)          # [P, S]
    sinT = np.ascontiguousarray(np.sin(freqs)[:, pidx].T * sign[:, None])

    pg = np.arange(P)[:, None, None]
    cg = np.arange(4)[None, :, None]
    fg = np.arange(QT)[None, None, :]
    maskT = ((fg - pg - 128 * cg) >= 0).astype(np.float32)

    xT = [np.ascontiguousarray(x[b].T) for b in range(B)]

    in_maps = []
    for c in range(N_CORES):
        b, k = c // 4, c % 4
        wkT_loc = np.concatenate(
            [np.tile(wkTf[:, HD * (2 * k + g):HD * (2 * k + g + 1)], (1, 2))
             for g in range(2)], axis=1)
        in_maps.append({
            "xT": xT[b],
            "wqT": np.ascontiguousarray(wqTf[:, 512 * k:512 * (k + 1)]),
            "wkT": np.ascontiguousarray(wkT_loc),
            "wvT": np.ascontiguousarray(wvTf[:, 128 * k:128 * (k + 1)]),
            "woT": np.ascontiguousarray(woTf[:, 512 * k:512 * (k + 1)]),
            "cosT": cosT,
            "sinT": sinT,
            "maskT": maskT,
        })
    return in_maps


LAST_EXEC_NS = None


def kernel(x, position_ids, wq, wk, wv, wo, _trace=False):
    from concourse import bass_utils

    if "nc" not in _CACHE:
        _CACHE["nc"] = _build()
    nc = _CACHE["nc"]

    in_maps = _prep_inputs(x, position_ids, wq, wk, wv, wo)
    res = bass_utils.run_bass_kernel_spmd(
        nc, in_maps, core_ids=list(range(N_CORES)), trace=_trace)

    global LAST_EXEC_NS
    LAST_EXEC_NS = res.exec_time_ns

    out = np.empty((B, S, DIM), dtype=np.float32)
    for c in range(N_CORES):
        b, k = c // 4, c % 4
        out[b, :, 512 * k:512 * (k + 1)] = res.results[c]["out_t"].T
    return out


# revision 8
# speedup vs baseline: 1.3024x; 1.3024x over previous
"""Trainium2 Bass kernel for causal GQA attention (B=2, S=2048, D=2048,
H=32, KVH=8, hd=64) with RoPE and output projection, running SPMD on 8
NeuronCores.

Sharding: tensor-parallel over heads (4-way) x data-parallel over batch
(2-way).  Core c (b = c//4, k = c%4) handles batch b and heads
8k..8k+8 (kv heads 2k, 2k+1).  Attention outputs are AllGathered within
each batch group of 4 cores (split per q-tile so the collectives overlap
attention and the wo matmuls), after which each core computes a 512-wide
output-dim slice of the wo projection.  The host assembles the full
output, so no AllReduce is needed.

Layouts: everything lives in transposed [feature, seq] form so that the
head dim (the contraction dim of QK^T) sits on SBUF partitions and no
on-device transposes are required (except a cheap PE transpose for V).
All matmuls run in float32r (fp32 storage, reduced-precision multiply,
1 cyc/row).
"""

import numpy as np

DIM = 2048
S = 2048
B = 2
H = 32
KVH = 8
HD = 64
P = 128
HL = 8          # heads per core
QT = 512        # q tile (free dim of score matmuls)
NQT = S // QT   # 4
NKV = S // P    # 16 kv tiles of 128
DK = DIM // P   # 16 contraction tiles
ROPE_BASE = 10000.0
N_CORES = 8

_CACHE = {}


def _build():
    import concourse.bacc as bacc
    import concourse.tile as tile
    import concourse.mybir as mybir
    from concourse.masks import make_identity

    F32 = mybir.dt.float32
    F32R = mybir.dt.float32r
    Exp = mybir.ActivationFunctionType.Exp

    nc = bacc.Bacc("TRN2", target_bir_lowering=False, debug=False,
                   num_devices=N_CORES)

    xT = nc.dram_tensor("xT", [DIM, S], F32R, kind="ExternalInput").ap()
    wqT = nc.dram_tensor("wqT", [DIM, 512], F32R, kind="ExternalInput").ap()
    wkT = nc.dram_tensor("wkT", [DIM, 256], F32R, kind="ExternalInput").ap()
    wvT = nc.dram_tensor("wvT", [DIM, 128], F32R, kind="ExternalInput").ap()
    woT = nc.dram_tensor("woT", [DIM, 512], F32R, kind="ExternalInput").ap()
    cosT = nc.dram_tensor("cosT", [P, S], F32, kind="ExternalInput").ap()
    sinT = nc.dram_tensor("sinT", [P, S], F32, kind="ExternalInput").ap()
    maskT = nc.dram_tensor("maskT", [P, 4, QT], F32, kind="ExternalInput").ap()
    out_t = nc.dram_tensor("out_t", [512, S], F32, kind="ExternalOutput").ap()

    xT3 = xT.rearrange("(o p) s -> p o s", p=P)
    wqT3 = wqT.rearrange("(o p) f -> p o f", p=P)
    wkT3 = wkT.rearrange("(o p) f -> p o f", p=P)
    wvT3 = wvT.rearrange("(o p) f -> p o f", p=P)
    woT3 = woT.rearrange("(o p) f -> p o f", p=P)

    with tile.TileContext(nc) as tc:
        with (
            tc.tile_pool(name="pers", bufs=1) as pers,
            tc.tile_pool(name="ps", bufs=1, space="PSUM") as ps,
            tc.tile_pool(name="dram", bufs=1, space="DRAM") as dram,
        ):
            # ---- persistent tiles ----
            q_fin = [pers.tile([P, S], F32R, name=f"q_fin{m}") for m in range(4)]
            k_fin = [pers.tile([P, S], F32R, name=f"k_fin{g}") for g in range(2)]
            v1 = [pers.tile([P, NKV, P], F32R, name=f"v1_{g}") for g in range(2)]
            msk = pers.tile([P, 4, QT], F32, name="msk")

            cc_in = [dram.tile([512, QT], F32R, name=f"cc_in{t}")
                     for t in range(NQT)]
            cc_out = [dram.tile([4 * 512, QT], F32R, name=f"cc_out{t}")
                      for t in range(NQT)]

            # PSUM layout (8 banks): tag sc2 = 2 tiles of 2 banks,
            # tag pv = 2 tiles of 1 bank, tag wo2 = 1 tile of 2 banks.
            def sc2(name):
                return ps.tile([P, 2, QT], F32, tag="sc2", bufs=2, name=name)

            def pvb(name, shape=None, dtype=None):
                return ps.tile(shape or [P, QT], dtype or F32, tag="pv",
                               bufs=2, name=name)

            def wo2(name):
                return ps.tile([P, 2, QT], F32, tag="wo2", bufs=1, name=name)

            # ================= stage A/B: projections + RoPE =================
            with tc.tile_pool(name="pa", bufs=1) as pa:
                OCH = 4  # contraction 128-tiles per x DMA chunk
                xsl0 = pa.tile([P, OCH, QT], F32R, tag="xsl", bufs=3,
                               name="xsl0")
                nc.sync.dma_start(xsl0[:], xT3[:, 0:OCH, 0:QT])
                wq_sb = [pa.tile([P, DK, P], F32R, name=f"wq_sb{m}")
                         for m in range(4)]
                for m in range(4):
                    nc.sync.dma_start(wq_sb[m][:],
                                      wqT3[:, :, m * P:(m + 1) * P])
                wk_sb = pa.tile([P, DK, 256], F32R)
                wv_sb = pa.tile([P, DK, 128], F32R)
                nc.sync.dma_start(wk_sb[:], wkT3[:])
                nc.sync.dma_start(wv_sb[:], wvT3[:])
                cos_sb = pa.tile([P, S], F32)
                sin_sb = pa.tile([P, S], F32)
                nc.sync.dma_start(cos_sb[:], cosT[:])
                nc.sync.dma_start(sin_sb[:], sinT[:])
                nc.sync.dma_start(msk[:], maskT[:])
                ident_f = pa.tile([P, P], F32)
                ident = pa.tile([P, P], F32R)
                make_identity(nc, ident_f[:])
                nc.vector.tensor_copy(ident[:], ident_f[:])

                vT_raw = pa.tile([P, S], F32R)

                # ones columns of the PV stationary operand (memset on f32r
                # is not a valid ISA encoding, so memset f32 then copy)
                ones3 = pa.tile([P, NKV, HD], F32)
                nc.vector.memset(ones3[:], 1.0)
                for g in range(2):
                    nc.vector.tensor_copy(v1[g][:, :, 0:HD], ones3[:])

                for st in range(NQT):
                    ssl = slice(st * QT, (st + 1) * QT)
                    qa = sc2(f"qa{st}")
                    qb = sc2(f"qb{st}")
                    kk0 = pvb(f"kk0{st}")
                    kk1 = pvb(f"kk1{st}")
                    vv = wo2(f"vv{st}")
                    qps = [qa[:, 0, :], qa[:, 1, :], qb[:, 0, :], qb[:, 1, :]]
                    kps = [kk0[:], kk1[:]]
                    vps = vv[:, 0, :]
                    for oc in range(DK // OCH):
                        if st == 0 and oc == 0:
                            xsl = xsl0
                        else:
                            xsl = pa.tile([P, OCH, QT], F32R, tag="xsl",
                                          bufs=3, name="xsl")
                            nc.sync.dma_start(
                                xsl[:], xT3[:, oc * OCH:(oc + 1) * OCH, ssl])
                        for oo in range(OCH):
                            o = oc * OCH + oo
                            first = o == 0
                            last = o == DK - 1
                            for m in range(4):
                                nc.tensor.matmul(
                                    qps[m], wq_sb[m][:, o, :], xsl[:, oo, :],
                                    start=first, stop=last)
                            for g in range(2):
                                nc.tensor.matmul(
                                    kps[g],
                                    wk_sb[:, o, g * P:(g + 1) * P],
                                    xsl[:, oo, :],
                                    start=first, stop=last)
                            nc.tensor.matmul(
                                vps, wv_sb[:, o, :], xsl[:, oo, :],
                                start=first, stop=last)

                    # RoPE on q/k slices; V^T raw copy
                    for dst, src in ([(q_fin[m], qps[m]) for m in range(4)]
                                     + [(k_fin[g], kps[g]) for g in range(2)]):
                        raw = pa.tile([P, QT], F32, tag="raw", bufs=3,
                                      name="raw")
                        nc.vector.tensor_copy(raw[:], src)
                        rot = pa.tile([P, QT], F32, tag="rot", bufs=3,
                                      name="rot")
                        for hh in range(2):
                            base = hh * HD
                            nc.sync.dma_start(rot[base:base + 32, :],
                                              raw[base + 32:base + 64, :])
                            nc.sync.dma_start(rot[base + 32:base + 64, :],
                                              raw[base:base + 32, :])
                        nc.vector.tensor_mul(rot[:], rot[:], sin_sb[:, ssl])
                        nc.vector.tensor_mul(raw[:], raw[:], cos_sb[:, ssl])
                        nc.vector.tensor_add(dst[:, ssl], raw[:], rot[:])
                    nc.vector.tensor_copy(vT_raw[:, ssl], vps)

                # V1 assembly: transpose vT_raw 128x128 blocks
                for j in range(NKV):
                    pst = pvb(f"pst{j}", [P, P], F32R)
                    nc.tensor.transpose(pst[:], vT_raw[:, j * P:(j + 1) * P],
                                        ident[:])
                    for g in range(2):
                        nc.vector.tensor_copy(
                            v1[g][:, j, HD:P], pst[:, g * HD:(g + 1) * HD])

            # ========== stage C/D/E: attention + AllGather + wo ==========
            with tc.tile_pool(name="pc", bufs=1) as pc:
                wo_sb = pc.tile([P, DK, 512], F32R, name="wo_sb")
                nc.sync.dma_start(wo_sb[:], woT3[:])
                for t in range(NQT):
                    ngrp = 2 * (t + 1)
                    qsl = slice(t * QT, (t + 1) * QT)
                    for h in range(HL):
                        m, half, g = h // 2, h % 2, h // 4
                        pr = slice(half * HD, half * HD + HD)
                        pspv = pvb(f"pv_{t}_{h}")
                        e_tiles = []
                        for g2 in range(ngrp):
                            pss = sc2(f"ss_{t}_{h}_{g2}")
                            for i in range(2):
                                j = 2 * g2 + i
                                nc.tensor.matmul(
                                    pss[:, i, :],
                                    k_fin[g][pr, j * P:(j + 1) * P],
                                    q_fin[m][pr, qsl],
                                    start=True, stop=True)
                            e2 = pc.tile([P, 2, QT], F32R, tag="exp", bufs=5,
                                         name="e2")
                            nc.scalar.activation(e2[:], pss[:], Exp,
                                                 scale=0.125)
                            cpair = g2 - 2 * t
                            if cpair >= 0:
                                nc.vector.tensor_mul(
                                    e2[:], e2[:],
                                    msk[:, 2 * cpair:2 * cpair + 2, :])
                            e_tiles.append(e2)
                        for g2 in range(ngrp):
                            for i in range(2):
                                j = 2 * g2 + i
                                nc.tensor.matmul(
                                    pspv[:], v1[g][:, j, :],
                                    e_tiles[g2][:, i, :],
                                    start=(j == 0), stop=(j == 4 * t + 3))
                        recip = pc.tile([1, QT], F32, tag="recip", bufs=2,
                                        name="recip")
                        nc.vector.reciprocal_approx_fast(recip[:],
                                                         pspv[0:1, :])
                        bcast = pc.tile([P, QT], F32, tag="bcast", bufs=2,
                                        name="bcast")
                        nc.gpsimd.partition_broadcast(bcast[:], recip[:])
                        o_sb = pc.tile([P, QT], F32R, tag="osb", bufs=2,
                                       name="o_sb")
                        nc.vector.tensor_mul(o_sb[HD:P, :], pspv[HD:P, :],
                                             bcast[HD:P, :])
                        nc.sync.dma_start(cc_in[t][h * HD:(h + 1) * HD, :],
                                          o_sb[HD:P, :])

                    # -------- AllGather for this q tile --------
                    nc.gpsimd.collective_compute(
                        "AllGather",
                        mybir.AluOpType.bypass,
                        replica_groups=[[0, 1, 2, 3], [4, 5, 6, 7]],
                        ins=[cc_in[t][:].opt()],
                        outs=[cc_out[t][:].opt()],
                    )
                    cc3 = cc_out[t][:].rearrange("(o p) s -> p o s", p=P)

                    # -------- wo projection for this q tile --------
                    cct = pc.tile([P, DK, QT], F32R, tag="cct", bufs=2,
                                  name="cct")
                    nc.sync.dma_start(cct[:], cc3[:])
                    for dp in range(2):
                        pw = wo2(f"wo_{t}_{dp}")
                        for o in range(DK):
                            for dd in range(2):
                                d = dp * 2 + dd
                                nc.tensor.matmul(
                                    pw[:, dd, :],
                                    wo_sb[:, o, d * P:(d + 1) * P],
                                    cct[:, o, :],
                                    start=(o == 0), stop=(o == DK - 1))
                        for dd in range(2):
                            d = dp * 2 + dd
                            ot = pc.tile([P, QT], F32, tag="ot", bufs=2,
                                         name="ot")
                            nc.vector.tensor_copy(ot[:], pw[:, dd, :])
                            nc.sync.dma_start(out_t[d * P:(d + 1) * P, qsl],
                                              ot[:])

    nc.compile()
    return nc


def _prep_inputs(x, position_ids, wq, wk, wv, wo):
    x = np.asarray(x, dtype=np.float32)
    pos = np.asarray(position_ids).reshape(-1).astype(np.int64)
    wqTf = np.asarray(wq, dtype=np.float32).T
    wkTf = np.asarray(wk, dtype=np.float32).T
    wvTf = np.asarray(wv, dtype=np.float32).T
    woTf = np.asarray(wo, dtype=np.float32).T

    inv = 1.0 / (ROPE_BASE ** (np.arange(0, HD, 2, dtype=np.float32) / HD))
    freqs = np.outer(pos.astype(np.float32), inv)  # [S, 32]
    pidx = np.arange(P) % 32
    sign = np.where((np.arange(P) % HD) < 32, -1.0, 1.0).astype(np.float32)
    cosT = np.ascontiguousarray(np.cos(freqs)[:, pidx].T)          # [P, S]
    sinT = np.ascontiguousarray(np.sin(freqs)[:, pidx].T * sign[:, None])

    pg = np.arange(P)[:, None, None]
    cg = np.arange(4)[None, :, None]
    fg = np.arange(QT)[None, None, :]
    maskT = ((fg - pg - 128 * cg) >= 0).astype(np.float32)

    xT = [np.ascontiguousarray(x[b].T) for b in range(B)]

    in_maps = []
    for c in range(N_CORES):
        b, k = c // 4, c % 4
        wkT_loc = np.concatenate(
            [np.tile(wkTf[:, HD * (2 * k + g):HD * (2 * k + g + 1)], (1, 2))
             for g in range(2)], axis=1)
        in_maps.append({
            "xT": xT[b],
            "wqT": np.ascontiguousarray(wqTf[:, 512 * k:512 * (k + 1)]),
            "wkT": np.ascontiguousarray(wkT_loc),
            "wvT": np.ascontiguousarray(wvTf[:, 128 * k:128 * (k + 1)]),
            "woT": np.ascontiguousarray(woTf[:, 512 * k:512 * (k + 1)]),
            "cosT": cosT,
            "sinT": sinT,
            "maskT": maskT,
        })
    return in_maps


LAST_EXEC_NS = None


def kernel(x, position_ids, wq, wk, wv, wo, _trace=False):
    from concourse import bass_utils

    if "nc" not in _CACHE:
        _CACHE["nc"] = _build()
    nc = _CACHE["nc"]

    in_maps = _prep_inputs(x, position_ids, wq, wk, wv, wo)
    res = bass_utils.run_bass_kernel_spmd(
        nc, in_maps, core_ids=list(range(N_CORES)), trace=_trace)

    global LAST_EXEC_NS
    LAST_EXEC_NS = res.exec_time_ns

    out = np.empty((B, S, DIM), dtype=np.float32)
    for c in range(N_CORES):
        b, k = c // 4, c % 4
        out[b, :, 512 * k:512 * (k + 1)] = res.results[c]["out_t"].T
    return out
